# revision 15
# baseline (speedup 1.0000x reference)
"""DISCO S2 convolution (nn_DISCOBlock_57801669869705) on 8 Trainium2 NeuronCores.

out[b,o,to,q] = sum_{c,k} w[o,c,k] * sum_{w,p} psi[k,to,w,p] * x[b,c,ti[to,w],(p+q)%P]

Mapping: for each output latitude row `to` and each active longitude-shift tap
(latitude-pair j, dp), one TensorE matmul accumulates into PSUM:
    out[:, (q,b)] += WPsi[(m,c), o].T @ xg[(m,c), (q+dp, b)]
with contraction over 128 partitions = (pair member m, channel c), M = o = 64,
N = (q,b) = 360. WPsi[(m,c), o] = sum_k psi[k,to,w(j,m),dp] * weight[o,c,k] is a
host-side transform of the small weight tensor; xg holds the latitudinally
gathered, longitudinally haloed input rows.

Precision/perf hybrid: taps are ranked by a data-independent psi-energy proxy.
The top HI_FRAC of taps run in bf16 (full-rate) matmuls; the low-energy tail
runs in fp8e4m3 using MatmulPerfMode.DoubleRow, which fuses TWO taps (two
K=128 contraction blocks) into a single PE pass — ~2x tap throughput. The two
fp8 blocks of a fused pair address arbitrary xg slices via a hand-built
3-level access pattern ([128, 2, 360] with a custom middle stride). HI and LO
accumulate in separate PSUM tiles; a single DVE scalar_tensor_tensor combines
them with the fp8 descale constant.

Sharding: the 91 output rows are grouped into 12 "slots" of <=8 rows; the rows
of a slot are computed simultaneously by the 8 cores (one row per core) under a
shared per-slot tap template (union of the rows' taps; absent taps get zero
coefficients). Grouping and the per-slot pairing of the 9 latitude-window rows
into 128-partition contraction blocks are jointly optimized by a DP to
minimize total matmul count.
"""

import math
from functools import lru_cache

import numpy as np

B, C, O = 2, 64, 64
NLAT, P = 91, 180
NR, NPHI = 5, 6
K = (NR - 1) * NPHI + 1
NCORE = 8
NSLOT = 12
NJ = 5  # pair slots per latitude window (4 pairs + 1 single)
# slot 11 (all-HI, tiny inputs) runs first so the PE starts while the bulk
# xg8/xgb/wp8 streams are still loading
SLOT_ORDER = [11] + list(range(11))
HI_FRAC = 0.20
SX = 32.0          # fp8 scale for xg (power of 2)
SW = 1048576.0     # fp8 scale for wp (power of 2), adjusted at runtime
WPH_CHUNK = 64     # HI taps per streamed weight-block DMA
WP8_CHUNK = 64     # LO DR-pairs per streamed weight-block DMA


def _compute_psi():
    theta_cut = 4.0 * math.pi / (NLAT - 1)
    half = int(math.ceil(theta_cut / (math.pi / (NLAT - 1))))
    theta = np.pi * np.arange(NLAT) / (NLAT - 1)
    phi_in = 2.0 * np.pi * np.arange(P) / P
    offs = np.arange(-half, half + 1)
    ti_raw = np.arange(NLAT)[:, None] + offs[None, :]
    valid = (ti_raw >= 0) & (ti_raw < NLAT)
    ti_idx = np.clip(ti_raw, 0, NLAT - 1)
    to = theta[:, None, None]
    ti = theta[ti_idx][:, :, None]
    ph = phi_in[None, None, :]
    xx = np.cos(to) * np.sin(ti) * np.cos(ph) - np.sin(to) * np.cos(ti)
    yy = np.sin(ti) * np.sin(ph)
    zz = np.sin(to) * np.sin(ti) * np.cos(ph) + np.cos(to) * np.cos(ti)
    r = np.arccos(np.clip(zz, -1.0, 1.0))
    az = np.mod(np.arctan2(yy, xx), 2.0 * np.pi)
    dr = theta_cut / (NR - 1)
    dphi = 2.0 * np.pi / NPHI
    inside = (r <= theta_cut) & valid[:, :, None]
    psi = np.zeros((K,) + r.shape)
    psi[0] = np.where(inside, np.maximum(0.0, 1.0 - r / dr), 0.0)
    for ir in range(1, NR):
        rad = np.maximum(0.0, 1.0 - np.abs(r - ir * dr) / dr)
        for ip in range(NPHI):
            d = np.abs(np.mod(az - ip * dphi + np.pi, 2.0 * np.pi) - np.pi)
            ang = np.maximum(0.0, 1.0 - d / dphi)
            psi[1 + (ir - 1) * NPHI + ip] = np.where(inside, rad * ang, 0.0)
    quad = np.sin(theta) * (np.pi / (NLAT - 1)) * (2.0 * np.pi / P)
    psi = psi * quad[ti_idx][None, :, :, None]
    return psi.astype(np.float32), ti_idx.astype(np.int32), 2 * half + 1


def _best_matching(u):
    """u: [W, P] bool. Return (cost, groups) — 4 pairs + 1 single over w=0..8
    minimizing sum over groups of |union of member activity|."""
    Wn = u.shape[0]
    M = np.zeros((Wn, Wn), dtype=np.int64)
    for a in range(Wn):
        for b in range(a + 1, Wn):
            M[a, b] = int((u[a] | u[b]).sum())
    s = np.array([int(u[w].sum()) for w in range(Wn)])
    INF = 10**12

    @lru_cache(maxsize=None)
    def f(mask, single_used):
        if mask == 0:
            return 0, ()
        a = (mask & -mask).bit_length() - 1
        rest = mask & ~(1 << a)
        best = (INF, ())
        for b in range(a + 1, Wn):
            if rest >> b & 1:
                c, pl = f(rest & ~(1 << b), single_used)
                if M[a, b] + c < best[0]:
                    best = (M[a, b] + c, pl + ((a, b),))
        if not single_used:
            c, pl = f(rest, True)
            if s[a] + c < best[0]:
                best = (s[a] + c, pl + ((a, None),))
        return best

    c, pl = f((1 << Wn) - 1, False)
    f.cache_clear()
    return c, list(pl)


def _build_plan():
    psi, ti_idx, W = _compute_psi()
    dpval = np.where(np.arange(P) < P // 2, np.arange(P), np.arange(P) - P)
    active = (psi != 0).any(axis=0)  # [To, W, P]

    cnt = active.reshape(NLAT, -1).sum(axis=1)
    order = list(np.argsort(-cnt, kind="stable"))
    n = len(order)
    INF = 10**12
    cost = np.full((n + 1, n + 1), INF, dtype=np.int64)
    pairings = {}
    for i in range(n):
        u = np.zeros((W, P), dtype=bool)
        for j in range(i + 1, min(i + 9, n + 1)):
            u = u | active[order[j - 1]]
            c, pl = _best_matching(u)
            cost[i][j] = c
            pairings[(i, j)] = pl
    dp = np.full((n + 1, NSLOT + 1), INF, dtype=np.int64)
    par = np.zeros((n + 1, NSLOT + 1), dtype=np.int64)
    dp[0][0] = 0
    for j in range(1, NSLOT + 1):
        for i in range(1, n + 1):
            for i0 in range(max(0, i - 8), i):
                v = dp[i0][j - 1] + cost[i0][i]
                if v < dp[i][j]:
                    dp[i][j] = v
                    par[i][j] = i0
    bounds = []
    i = n
    for j in range(NSLOT, 0, -1):
        i0 = par[i][j]
        bounds.append((i0, i))
        i = i0
    bounds = bounds[::-1]

    row_of = -np.ones((NCORE, NSLOT), dtype=np.int64)
    slot_pairs, templates, halos = [], [], []
    for s, (i0, i1) in enumerate(bounds):
        rows = [order[t] for t in range(i0, i1)]
        for ci, t in enumerate(rows):
            row_of[ci, s] = t
        pairs = pairings[(i0, i1)]
        assert len(pairs) == NJ
        slot_pairs.append(pairs)
        u = active[rows].any(axis=0)  # [W, P]
        tap_list = []
        for j, (wa, wb) in enumerate(pairs):
            ws = [w for w in (wa, wb) if w is not None]
            act_j = u[ws].any(axis=0)  # [P]
            pp = np.nonzero(act_j)[0]
            for dp_ in sorted(dpval[pp].tolist()):
                tap_list.append((j, dp_))
        templates.append(tap_list)
        halos.append(max((abs(d) for _, d in tap_list), default=0))

    qpads = [P + 2 * h for h in halos]
    offs = np.cumsum([0] + [NJ * B * qp for qp in qpads]).tolist()
    T = int(sum(len(t) for t in templates))

    # ---- data-independent per-tap energy proxy: sum over cores/members of
    # ||psi[:, to, w, p]||^2 (proportional to E||WPsi_t||^2 for iid weight) ----
    E = np.zeros(T)
    tg = 0
    for s in range(NSLOT):
        pairs = slot_pairs[s]
        for (j, dp_) in templates[s]:
            p = dp_ % P
            e = 0.0
            for core in range(NCORE):
                to = row_of[core, s]
                if to < 0:
                    continue
                for w_ in pairs[j]:
                    if w_ is not None:
                        e += float((psi[:, to, w_, p].astype(np.float64) ** 2).sum())
            E[tg] = e
            tg += 1
    n_hi = int(HI_FRAC * T)
    hi_mask = np.zeros(T, dtype=bool)
    hi_mask[np.argsort(-E)[:n_hi]] = True

    # ---- per-slot HI lists and LO pairs (slot 0 pairs stay within one j so
    # its xg8 can be split into per-j tiles for a fast start) ----
    slot_hi, slot_lo_pairs = [], []
    tg = 0
    for s in range(NSLOT):
        his, los = [], []
        for (j, dp_) in templates[s]:
            (his if hi_mask[tg] else los).append((j, dp_, tg))
            tg += 1
        pairs_lo = []
        if s == 0:
            byj = {}
            for t in los:
                byj.setdefault(t[0], []).append(t)
            for j in sorted(byj):
                lst = byj[j]
                for i in range(0, len(lst) - 1, 2):
                    pairs_lo.append((lst[i], lst[i + 1]))
                if len(lst) % 2:
                    pairs_lo.append((lst[-1], None))
        else:
            for i in range(0, len(los) - 1, 2):
                pairs_lo.append((los[i], los[i + 1]))
            if len(los) % 2:
                pairs_lo.append((los[-1], None))
        slot_hi.append(his)
        slot_lo_pairs.append(pairs_lo)

    n_hi_total = int(sum(len(h) for h in slot_hi))
    n_dr_total = int(sum(len(p) for p in slot_lo_pairs))

    return dict(psi=psi, ti_idx=ti_idx, W=W, row_of=row_of, templates=templates,
                slot_pairs=slot_pairs, halos=halos, qpads=qpads, offs=offs,
                xg_cols=int(offs[-1]), t_total=T, hi_mask=hi_mask,
                slot_hi=slot_hi, slot_lo_pairs=slot_lo_pairs,
                n_hi=n_hi_total, n_dr=n_dr_total)


_PLAN = None
_NC = None


def _get_plan():
    global _PLAN
    if _PLAN is None:
        _PLAN = _build_plan()
    return _PLAN


def _build_nc(plan):
    import concourse.bacc as bacc
    import concourse.mybir as mybir
    import concourse.tile as tile

    f32 = mybir.dt.float32
    bf16 = mybir.dt.bfloat16
    f8 = mybir.dt.float8e4
    DR = mybir.MatmulPerfMode.DoubleRow
    DSC = 1.0 / (SX * SW)

    halos = plan["halos"]
    qpads = plan["qpads"]
    offs = plan["offs"]
    XG_COLS = plan["xg_cols"]
    slot_hi = plan["slot_hi"]
    slot_lo_pairs = plan["slot_lo_pairs"]
    N_HI = max(plan["n_hi"], 1)
    N_DR = max(plan["n_dr"], 1)

    nc = bacc.Bacc("TRN2", target_bir_lowering=False, debug=False,
                   num_devices=NCORE)
    xgb_d = nc.declare_dram_parameter("xgb", [128, XG_COLS], bf16,
                                      isOutput=False)
    xg8_d = nc.declare_dram_parameter("xg8", [128, XG_COLS], f8,
                                      isOutput=False)
    wph_d = nc.declare_dram_parameter("wph", [128, N_HI * O], bf16,
                                      isOutput=False)
    wp8_d = nc.declare_dram_parameter("wp8", [128, N_DR * 128], f8,
                                      isOutput=False)
    out_d = nc.declare_dram_parameter("out", [O, NSLOT * B * P], f32,
                                      isOutput=True)

    with tile.TileContext(nc) as tc:
        with (
            tc.tile_pool(name="xgb", bufs=1) as xgbp,
            tc.tile_pool(name="xg8", bufs=1) as xg8p,
            tc.tile_pool(name="wph", bufs=3) as wphp,
            tc.tile_pool(name="wp8", bufs=4) as wp8p,
            tc.tile_pool(name="ps", bufs=4, space="PSUM") as psp,
            tc.tile_pool(name="outp", bufs=2) as outp,
        ):
            # DMA queues: scalar (HWDGE) carries the first slot's xgb plus all
            # of xg8 (needed early, small); gpsimd (SWDGE) carries the bulk
            # bf16 xgb; sync carries the weight chunk streams + output. xg8
            # slot 0 is split per pair-slot j so slot 0's first DR matmul
            # waits on a small piece only.
            s_first = SLOT_ORDER[0]
            xgb_ts = [None] * NSLOT
            seg_f = xgbp.tile([128, NJ * B * qpads[s_first]], bf16,
                              name="xgbf", tag=f"xgb{s_first}")
            nc.scalar.dma_start(seg_f[:], xgb_d[:, offs[s_first]:
                                                offs[s_first + 1]])
            xgb_ts[s_first] = seg_f
            xg8_ts = [None] * NSLOT
            qp0 = qpads[0]
            pieces = []
            for j in range(NJ):
                pj = xg8p.tile([128, B * qp0], f8, tag=f"xg8_0_{j}")
                nc.scalar.dma_start(
                    pj[:], xg8_d[:, offs[0] + j * B * qp0:
                                 offs[0] + (j + 1) * B * qp0])
                pieces.append(pj)
            xg8_ts[0] = pieces
            for s in range(1, NSLOT):
                seg = xg8p.tile([128, NJ * B * qpads[s]], f8, tag=f"xg8_{s}")
                nc.scalar.dma_start(seg[:], xg8_d[:, offs[s]:offs[s + 1]])
                xg8_ts[s] = seg
            for s in range(NSLOT):
                if s == s_first:
                    continue
                seg = xgbp.tile([128, NJ * B * qpads[s]], bf16, name="xgbs",
                                tag=f"xgb{s}")
                nc.gpsimd.dma_start(seg[:], xgb_d[:, offs[s]:offs[s + 1]])
                xgb_ts[s] = seg
            out_t = outp.tile([O, NSLOT * B * P], f32)

            # chunk boundaries for the two weight streams (small first chunks
            # shorten the preamble)
            def chunk_bounds(total, grades, step):
                bnds = [0]
                for g in grades:
                    if bnds[-1] < total:
                        bnds.append(min(total, bnds[-1] + g))
                while bnds[-1] < total:
                    bnds.append(min(total, bnds[-1] + step))
                cof = []
                for ci in range(len(bnds) - 1):
                    cof += [(ci, bnds[ci])] * (bnds[ci + 1] - bnds[ci])
                return bnds, cof

            hb, hc = chunk_bounds(plan["n_hi"], [8, 16, 32], WPH_CHUNK)
            lb, lc = chunk_bounds(plan["n_dr"], [8, 16, 32], WP8_CHUNK)

            gh = 0  # global HI tap index
            gl = 0  # global DR pair index
            wph_t = None
            wp8_t = None
            for s in SLOT_ORDER:
                his = slot_hi[s]
                lops = slot_lo_pairs[s]
                qp = qpads[s]
                h = halos[s]

                def xoff(j, dp):
                    return j * B * qp + B * (h + dp)

                acc_h = (psp.tile([O, B * P], f32, name="acch", tag="acch")
                         if his else None)
                acc_l = (psp.tile([O, B * P], f32, name="accl", tag="accl")
                         if lops else None)
                for i, (j, dp, _) in enumerate(his):
                    cidx, cbase = hc[gh]
                    if gh == cbase:
                        cols = (hb[cidx + 1] - cbase) * O
                        wph_t = wphp.tile([128, WPH_CHUNK * O], bf16, tag="wph")
                        nc.sync.dma_start(
                            wph_t[:, :cols], wph_d[:, cbase * O:cbase * O + cols])
                    lhsT = wph_t[:, (gh - cbase) * O:(gh - cbase + 1) * O]
                    rhs = xgb_ts[s][:, xoff(j, dp):xoff(j, dp) + B * P]
                    nc.tensor.matmul(acc_h[:], lhsT, rhs,
                                     start=(i == 0), stop=(i == len(his) - 1))
                    gh += 1
                for i, (ta, tb) in enumerate(lops):
                    cidx, cbase = lc[gl]
                    if gl == cbase:
                        cols = (lb[cidx + 1] - cbase) * 128
                        wp8_t = wp8p.tile([128, WP8_CHUNK * 128], f8, tag="wp8")
                        nc.sync.dma_start(
                            wp8_t[:, :cols],
                            wp8_d[:, cbase * 128:cbase * 128 + cols])
                    lhsT = wp8_t[:, (gl - cbase) * 128:(gl - cbase + 1) * 128]
                    lhsT3 = lhsT.rearrange("p (two m) -> p two m", two=2)
                    ja, dpa, _ = ta
                    if s == 0:
                        # slot-0 pairs stay within pair-slot ja's piece
                        xv = xg8_ts[0][ja]
                        offa = B * (h + dpa)
                        offb = offa if tb is None else B * (h + tb[1])
                    else:
                        xv = xg8_ts[s]
                        offa = xoff(ja, dpa)
                        offb = offa if tb is None else xoff(tb[0], tb[1])
                    rhs3 = xv[:, offa:offa + B * P].unsqueeze(1)
                    rhs3.ap[1] = [offb - offa, 2]
                    nc.tensor.matmul(acc_l[:], lhsT3, rhs3,
                                     start=(i == 0), stop=(i == len(lops) - 1),
                                     perf_mode=DR)
                    gl += 1
                dst = out_t[:, s * B * P:(s + 1) * B * P]
                if his and lops:
                    # TensorScalarPtr may read only one input from PSUM
                    nc.vector.tensor_copy(dst, acc_h[:])
                    nc.vector.scalar_tensor_tensor(
                        dst, acc_l[:], DSC, dst,
                        op0=_alu_mult(), op1=_alu_add())
                elif his:
                    nc.vector.tensor_copy(dst, acc_h[:])
                else:
                    nc.vector.tensor_scalar_mul(dst, acc_l[:], DSC)
                nc.sync.dma_start(
                    out_d[:, s * B * P:(s + 1) * B * P], dst)

    nc.compile()
    return nc


def _alu_mult():
    import concourse.mybir as mybir
    return mybir.AluOpType.mult


def _alu_add():
    import concourse.mybir as mybir
    return mybir.AluOpType.add


def _get_nc():
    global _NC
    if _NC is None:
        _NC = _build_nc(_get_plan())
    return _NC


def _build_core_inputs(plan, x, weight):
    import ml_dtypes

    psi = plan["psi"]
    ti_idx = plan["ti_idx"]
    row_of = plan["row_of"]
    templates = plan["templates"]
    slot_pairs = plan["slot_pairs"]
    halos = plan["halos"]
    qpads = plan["qpads"]
    offs = plan["offs"]
    XG_COLS = plan["xg_cols"]
    T = plan["t_total"]
    slot_hi = plan["slot_hi"]
    slot_lo_pairs = plan["slot_lo_pairs"]
    N_HI = max(plan["n_hi"], 1)
    N_DR = max(plan["n_dr"], 1)

    f8 = ml_dtypes.float8_e4m3
    bf16 = ml_dtypes.bfloat16

    coef = np.zeros((NCORE, T, 2, K), dtype=np.float32)
    tg = 0
    for s in range(NSLOT):
        pairs = slot_pairs[s]
        for (j, dp) in templates[s]:
            p = dp % P
            members = pairs[j]
            for core in range(NCORE):
                to = row_of[core, s]
                if to < 0:
                    continue
                for m in range(2):
                    w_ = members[m] if m < len(members) else None
                    if w_ is not None:
                        coef[core, tg, m] = psi[:, to, w_, p]
            tg += 1
    wk = np.ascontiguousarray(weight.transpose(2, 1, 0)).reshape(K, C, O)
    wp_all = np.einsum("ntmk,kco->ntmco", coef, wk, optimize=True)
    # wp_all[n, t] : [2, C, O] -> [128, O] per tap

    in_maps = []
    for core in range(NCORE):
        wp = np.ascontiguousarray(
            wp_all[core].transpose(1, 2, 0, 3).reshape(128, T * O))
        wph = np.zeros((128, N_HI * O), dtype=bf16)
        wp8 = np.zeros((128, N_DR * 128), dtype=f8)
        gh = 0
        gl = 0
        for s in SLOT_ORDER:
            for (_, _, tg_) in slot_hi[s]:
                wph[:, gh * O:(gh + 1) * O] = wp[:, tg_ * O:(tg_ + 1) * O]
                gh += 1
            for (ta, tb) in slot_lo_pairs[s]:
                blk = np.zeros((128, 128), dtype=np.float32)
                blk[:, :O] = wp[:, ta[2] * O:(ta[2] + 1) * O] * SW
                if tb is not None:
                    blk[:, O:] = wp[:, tb[2] * O:(tb[2] + 1) * O] * SW
                wp8[:, gl * 128:(gl + 1) * 128] = np.clip(
                    blk, -240.0, 240.0).astype(f8)
                gl += 1

        xg = np.zeros((128, XG_COLS), dtype=np.float32)
        for s in range(NSLOT):
            to = row_of[core, s]
            if to < 0:
                continue
            qp = qpads[s]
            h = halos[s]
            qq = (np.arange(qp) - h) % P
            for j, members in enumerate(slot_pairs[s]):
                for m in range(2):
                    w_ = members[m] if m < len(members) else None
                    if w_ is None:
                        continue
                    ti = ti_idx[to, w_]
                    blk = x[:, :, ti, :][:, :, qq]  # [b, c, qp]
                    # column layout (qq, b) so a dp-shifted rhs is contiguous
                    xg[m * 64:(m + 1) * 64,
                       offs[s] + j * B * qp: offs[s] + (j + 1) * B * qp] = (
                        blk.transpose(1, 2, 0).reshape(C, qp * B))
        xgb = xg.astype(bf16)
        xg8 = np.clip(xg * SX, -240.0, 240.0).astype(f8)
        in_maps.append({"xgb": xgb, "xg8": xg8, "wph": wph, "wp8": wp8})
    return in_maps


def kernel(x, weight):
    from concourse.bass_utils import run_bass_kernel_spmd

    x = np.ascontiguousarray(np.asarray(x, dtype=np.float32))
    weight = np.ascontiguousarray(np.asarray(weight, dtype=np.float32))
    plan = _get_plan()
    nc = _get_nc()
    in_maps = _build_core_inputs(plan, x, weight)
    res = run_bass_kernel_spmd(nc, in_maps, list(range(NCORE)))

    out = np.zeros((B, O, NLAT, P), dtype=np.float32)
    row_of = plan["row_of"]
    for core in range(NCORE):
        oc = np.asarray(res.results[core]["out"]).reshape(O, NSLOT, P, B)
        for s in range(NSLOT):
            to = row_of[core, s]
            if to >= 0:
                out[:, :, to, :] = oc[:, s, :, :].transpose(2, 0, 1)
    return out


def _numpy_sim(x, weight):
    """Host replica of the device program (for validation)."""
    plan = _get_plan()
    in_maps = _build_core_inputs(plan, x, weight)
    halos = plan["halos"]
    qpads = plan["qpads"]
    slot_hi = plan["slot_hi"]
    slot_lo_pairs = plan["slot_lo_pairs"]
    offs = plan["offs"]
    out = np.zeros((B, O, NLAT, P), dtype=np.float32)
    row_of = plan["row_of"]
    for core in range(NCORE):
        m = in_maps[core]
        xgb = m["xgb"].astype(np.float32)
        xg8 = m["xg8"].astype(np.float32)
        wph = m["wph"].astype(np.float32)
        wp8 = m["wp8"].astype(np.float32)
        gh = 0
        gl = 0
        oc = np.zeros((O, NSLOT, P, B), dtype=np.float32)
        for s in SLOT_ORDER:
            qp = qpads[s]
            h = halos[s]
            acc_h = np.zeros((O, P * B), dtype=np.float32)
            acc_l = np.zeros((O, P * B), dtype=np.float32)
            for (j, dp, _) in slot_hi[s]:
                off = offs[s] + j * B * qp + B * (h + dp)
                acc_h += wph[:, gh * O:(gh + 1) * O].T @ xgb[:, off:off + B * P]
                gh += 1
            for (ta, tb) in slot_lo_pairs[s]:
                lhsT = wp8[:, gl * 128:(gl + 1) * 128]
                offa = offs[s] + ta[0] * B * qp + B * (h + ta[1])
                acc_l += lhsT[:, :O].T @ xg8[:, offa:offa + B * P]
                if tb is not None:
                    offb = offs[s] + tb[0] * B * qp + B * (h + tb[1])
                    acc_l += lhsT[:, O:].T @ xg8[:, offb:offb + B * P]
                gl += 1
            oc[:, s] = (acc_h + acc_l / (SX * SW)).reshape(O, P, B)
        for s in range(NSLOT):
            to = row_of[core, s]
            if to >= 0:
                out[:, :, to, :] = oc[:, s, :, :].transpose(2, 0, 1)
    return out


if __name__ == "__main__":
    plan = _get_plan()
    print("t_total:", plan["t_total"], "n_hi:", plan["n_hi"],
          "n_dr:", plan["n_dr"], "xg_cols:", plan["xg_cols"])
    d = np.load("/tmp/ref_io.npz")
    got = _numpy_sim(d["x"], d["weight"])
    exp = d["expected"]
    rel = np.linalg.norm((got - exp).ravel()) / np.linalg.norm(exp.ravel())
    print("numpy-sim rel err:", rel)


# revision 16
# speedup vs baseline: 1.0897x; 1.0897x over previous
"""DISCO S2 convolution (nn_DISCOBlock_57801669869705) on 8 Trainium2 NeuronCores.

out[b,o,to,q] = sum_{c,k} w[o,c,k] * sum_{w,p} psi[k,to,w,p] * x[b,c,ti[to,w],(p+q)%P]

Mapping: for each output latitude row `to` and each active longitude-shift tap
(latitude-pair j, dp), one TensorE matmul accumulates into PSUM:
    out[:, (q,b)] += WPsi[(m,c), o].T @ xg[(m,c), (q+dp, b)]
with contraction over 128 partitions = (pair member m, channel c), M = o = 64,
N = (q,b) = 360. WPsi[(m,c), o] = sum_k psi[k,to,w(j,m),dp] * weight[o,c,k] is a
host-side transform of the small weight tensor; xg holds the latitudinally
gathered, longitudinally haloed input rows.

Precision/perf hybrid: taps are ranked by a data-independent psi-energy proxy.
The top HI_FRAC of taps run in bf16 (full-rate) matmuls; the low-energy tail
runs in fp8e4m3 using MatmulPerfMode.DoubleRow, which fuses TWO taps (two
K=128 contraction blocks) into a single PE pass — ~2x tap throughput. The two
fp8 blocks of a fused pair address arbitrary xg slices via a hand-built
3-level access pattern ([128, 2, 360] with a custom middle stride). HI and LO
accumulate in separate PSUM tiles; a single DVE scalar_tensor_tensor combines
them with the fp8 descale constant.

Sharding: the 91 output rows are grouped into 12 "slots" of <=8 rows; the rows
of a slot are computed simultaneously by the 8 cores (one row per core) under a
shared per-slot tap template (union of the rows' taps; absent taps get zero
coefficients). Grouping and the per-slot pairing of the 9 latitude-window rows
into 128-partition contraction blocks are jointly optimized by a DP to
minimize total matmul count.
"""

import math
from functools import lru_cache

import numpy as np

B, C, O = 2, 64, 64
NLAT, P = 91, 180
NR, NPHI = 5, 6
K = (NR - 1) * NPHI + 1
NCORE = 8
NSLOT = 12
NJ = 5  # pair slots per latitude window (4 pairs + 1 single)
# slot 11 (all-HI, tiny inputs) runs first so the PE starts while the bulk
# xg8/xgb/wp8 streams are still loading
SLOT_ORDER = [11] + list(range(11))
HI_FRAC = 0.20
SX = 32.0          # fp8 scale for xg (power of 2)
SW = 1048576.0     # fp8 scale for wp (power of 2), adjusted at runtime
WPH_CHUNK = 64     # HI taps per streamed weight-block DMA
WP8_CHUNK = 64     # LO DR-pairs per streamed weight-block DMA


def _compute_psi():
    theta_cut = 4.0 * math.pi / (NLAT - 1)
    half = int(math.ceil(theta_cut / (math.pi / (NLAT - 1))))
    theta = np.pi * np.arange(NLAT) / (NLAT - 1)
    phi_in = 2.0 * np.pi * np.arange(P) / P
    offs = np.arange(-half, half + 1)
    ti_raw = np.arange(NLAT)[:, None] + offs[None, :]
    valid = (ti_raw >= 0) & (ti_raw < NLAT)
    ti_idx = np.clip(ti_raw, 0, NLAT - 1)
    to = theta[:, None, None]
    ti = theta[ti_idx][:, :, None]
    ph = phi_in[None, None, :]
    xx = np.cos(to) * np.sin(ti) * np.cos(ph) - np.sin(to) * np.cos(ti)
    yy = np.sin(ti) * np.sin(ph)
    zz = np.sin(to) * np.sin(ti) * np.cos(ph) + np.cos(to) * np.cos(ti)
    r = np.arccos(np.clip(zz, -1.0, 1.0))
    az = np.mod(np.arctan2(yy, xx), 2.0 * np.pi)
    dr = theta_cut / (NR - 1)
    dphi = 2.0 * np.pi / NPHI
    inside = (r <= theta_cut) & valid[:, :, None]
    psi = np.zeros((K,) + r.shape)
    psi[0] = np.where(inside, np.maximum(0.0, 1.0 - r / dr), 0.0)
    for ir in range(1, NR):
        rad = np.maximum(0.0, 1.0 - np.abs(r - ir * dr) / dr)
        for ip in range(NPHI):
            d = np.abs(np.mod(az - ip * dphi + np.pi, 2.0 * np.pi) - np.pi)
            ang = np.maximum(0.0, 1.0 - d / dphi)
            psi[1 + (ir - 1) * NPHI + ip] = np.where(inside, rad * ang, 0.0)
    quad = np.sin(theta) * (np.pi / (NLAT - 1)) * (2.0 * np.pi / P)
    psi = psi * quad[ti_idx][None, :, :, None]
    return psi.astype(np.float32), ti_idx.astype(np.int32), 2 * half + 1


def _best_matching(u):
    """u: [W, P] bool. Return (cost, groups) — 4 pairs + 1 single over w=0..8
    minimizing sum over groups of |union of member activity|."""
    Wn = u.shape[0]
    M = np.zeros((Wn, Wn), dtype=np.int64)
    for a in range(Wn):
        for b in range(a + 1, Wn):
            M[a, b] = int((u[a] | u[b]).sum())
    s = np.array([int(u[w].sum()) for w in range(Wn)])
    INF = 10**12

    @lru_cache(maxsize=None)
    def f(mask, single_used):
        if mask == 0:
            return 0, ()
        a = (mask & -mask).bit_length() - 1
        rest = mask & ~(1 << a)
        best = (INF, ())
        for b in range(a + 1, Wn):
            if rest >> b & 1:
                c, pl = f(rest & ~(1 << b), single_used)
                if M[a, b] + c < best[0]:
                    best = (M[a, b] + c, pl + ((a, b),))
        if not single_used:
            c, pl = f(rest, True)
            if s[a] + c < best[0]:
                best = (s[a] + c, pl + ((a, None),))
        return best

    c, pl = f((1 << Wn) - 1, False)
    f.cache_clear()
    return c, list(pl)


def _build_plan():
    psi, ti_idx, W = _compute_psi()
    dpval = np.where(np.arange(P) < P // 2, np.arange(P), np.arange(P) - P)
    active = (psi != 0).any(axis=0)  # [To, W, P]

    cnt = active.reshape(NLAT, -1).sum(axis=1)
    order = list(np.argsort(-cnt, kind="stable"))
    n = len(order)
    INF = 10**12
    cost = np.full((n + 1, n + 1), INF, dtype=np.int64)
    pairings = {}
    for i in range(n):
        u = np.zeros((W, P), dtype=bool)
        for j in range(i + 1, min(i + 9, n + 1)):
            u = u | active[order[j - 1]]
            c, pl = _best_matching(u)
            cost[i][j] = c
            pairings[(i, j)] = pl
    dp = np.full((n + 1, NSLOT + 1), INF, dtype=np.int64)
    par = np.zeros((n + 1, NSLOT + 1), dtype=np.int64)
    dp[0][0] = 0
    for j in range(1, NSLOT + 1):
        for i in range(1, n + 1):
            for i0 in range(max(0, i - 8), i):
                v = dp[i0][j - 1] + cost[i0][i]
                if v < dp[i][j]:
                    dp[i][j] = v
                    par[i][j] = i0
    bounds = []
    i = n
    for j in range(NSLOT, 0, -1):
        i0 = par[i][j]
        bounds.append((i0, i))
        i = i0
    bounds = bounds[::-1]

    row_of = -np.ones((NCORE, NSLOT), dtype=np.int64)
    slot_pairs, templates, halos = [], [], []
    for s, (i0, i1) in enumerate(bounds):
        rows = [order[t] for t in range(i0, i1)]
        for ci, t in enumerate(rows):
            row_of[ci, s] = t
        pairs = pairings[(i0, i1)]
        assert len(pairs) == NJ
        slot_pairs.append(pairs)
        u = active[rows].any(axis=0)  # [W, P]
        tap_list = []
        for j, (wa, wb) in enumerate(pairs):
            ws = [w for w in (wa, wb) if w is not None]
            act_j = u[ws].any(axis=0)  # [P]
            pp = np.nonzero(act_j)[0]
            for dp_ in sorted(dpval[pp].tolist()):
                tap_list.append((j, dp_))
        templates.append(tap_list)
        halos.append(max((abs(d) for _, d in tap_list), default=0))

    qpads = [P + 2 * h for h in halos]
    offs = np.cumsum([0] + [NJ * B * qp for qp in qpads]).tolist()
    T = int(sum(len(t) for t in templates))

    # ---- data-independent per-tap energy proxy: sum over cores/members of
    # ||psi[:, to, w, p]||^2 (proportional to E||WPsi_t||^2 for iid weight) ----
    E = np.zeros(T)
    tg = 0
    for s in range(NSLOT):
        pairs = slot_pairs[s]
        for (j, dp_) in templates[s]:
            p = dp_ % P
            e = 0.0
            for core in range(NCORE):
                to = row_of[core, s]
                if to < 0:
                    continue
                for w_ in pairs[j]:
                    if w_ is not None:
                        e += float((psi[:, to, w_, p].astype(np.float64) ** 2).sum())
            E[tg] = e
            tg += 1
    n_hi = int(HI_FRAC * T)
    hi_mask = np.zeros(T, dtype=bool)
    hi_mask[np.argsort(-E)[:n_hi]] = True

    # ---- per-slot HI lists and LO pairs (slot 0 pairs stay within one j so
    # its xg8 can be split into per-j tiles for a fast start) ----
    slot_hi, slot_lo_pairs = [], []
    tg = 0
    for s in range(NSLOT):
        his, los = [], []
        for (j, dp_) in templates[s]:
            (his if hi_mask[tg] else los).append((j, dp_, tg))
            tg += 1
        pairs_lo = []
        if s == 0:
            byj = {}
            for t in los:
                byj.setdefault(t[0], []).append(t)
            for j in sorted(byj):
                lst = byj[j]
                for i in range(0, len(lst) - 1, 2):
                    pairs_lo.append((lst[i], lst[i + 1]))
                if len(lst) % 2:
                    pairs_lo.append((lst[-1], None))
        else:
            for i in range(0, len(los) - 1, 2):
                pairs_lo.append((los[i], los[i + 1]))
            if len(los) % 2:
                pairs_lo.append((los[-1], None))
        slot_hi.append(his)
        slot_lo_pairs.append(pairs_lo)

    n_hi_total = int(sum(len(h) for h in slot_hi))
    n_dr_total = int(sum(len(p) for p in slot_lo_pairs))

    return dict(psi=psi, ti_idx=ti_idx, W=W, row_of=row_of, templates=templates,
                slot_pairs=slot_pairs, halos=halos, qpads=qpads, offs=offs,
                xg_cols=int(offs[-1]), t_total=T, hi_mask=hi_mask,
                slot_hi=slot_hi, slot_lo_pairs=slot_lo_pairs,
                n_hi=n_hi_total, n_dr=n_dr_total)


_PLAN = None
_NC = None


def _get_plan():
    global _PLAN
    if _PLAN is None:
        _PLAN = _build_plan()
    return _PLAN


def _build_nc(plan):
    import concourse.bacc as bacc
    import concourse.mybir as mybir
    import concourse.tile as tile

    f32 = mybir.dt.float32
    bf16 = mybir.dt.bfloat16
    f8 = mybir.dt.float8e4
    DR = mybir.MatmulPerfMode.DoubleRow
    DSC = 1.0 / (SX * SW)

    halos = plan["halos"]
    qpads = plan["qpads"]
    offs = plan["offs"]
    XG_COLS = plan["xg_cols"]
    slot_hi = plan["slot_hi"]
    slot_lo_pairs = plan["slot_lo_pairs"]
    N_HI = max(plan["n_hi"], 1)
    N_DR = max(plan["n_dr"], 1)

    nc = bacc.Bacc("TRN2", target_bir_lowering=False, debug=False,
                   num_devices=NCORE)
    xgb_d = nc.declare_dram_parameter("xgb", [128, XG_COLS], bf16,
                                      isOutput=False)
    xg8_d = nc.declare_dram_parameter("xg8", [128, XG_COLS], f8,
                                      isOutput=False)
    wph_d = nc.declare_dram_parameter("wph", [128, N_HI * O], bf16,
                                      isOutput=False)
    wp8_d = nc.declare_dram_parameter("wp8", [128, N_DR * 128], f8,
                                      isOutput=False)
    out_d = nc.declare_dram_parameter("out", [O, NSLOT * B * P], f32,
                                      isOutput=True)

    with tile.TileContext(nc) as tc:
        with (
            tc.tile_pool(name="xgb", bufs=1) as xgbp,
            tc.tile_pool(name="xg8", bufs=1) as xg8p,
            tc.tile_pool(name="wph", bufs=3) as wphp,
            tc.tile_pool(name="wp8", bufs=4) as wp8p,
            tc.tile_pool(name="ps", bufs=4, space="PSUM") as psp,
            tc.tile_pool(name="outp", bufs=2) as outp,
        ):
            # DMA queues: scalar (HWDGE) carries the first slot's xgb plus all
            # of xg8 (needed early, small); gpsimd (SWDGE) carries the bulk
            # bf16 xgb; sync carries the weight chunk streams + output. xg8
            # slot 0 is split per pair-slot j so slot 0's first DR matmul
            # waits on a small piece only.
            s_first = SLOT_ORDER[0]
            xgb_ts = [None] * NSLOT
            seg_f = xgbp.tile([128, NJ * B * qpads[s_first]], bf16,
                              name="xgbf", tag=f"xgb{s_first}")
            nc.scalar.dma_start(seg_f[:], xgb_d[:, offs[s_first]:
                                                offs[s_first + 1]])
            xgb_ts[s_first] = seg_f
            xg8_ts = [None] * NSLOT
            qp0 = qpads[0]
            pieces = []
            for j in range(NJ):
                pj = xg8p.tile([128, B * qp0], f8, tag=f"xg8_0_{j}")
                nc.gpsimd.dma_start(
                    pj[:], xg8_d[:, offs[0] + j * B * qp0:
                                 offs[0] + (j + 1) * B * qp0])
                pieces.append(pj)
            xg8_ts[0] = pieces
            for s in range(1, NSLOT):
                seg = xg8p.tile([128, NJ * B * qpads[s]], f8, tag=f"xg8_{s}")
                nc.gpsimd.dma_start(seg[:], xg8_d[:, offs[s]:offs[s + 1]])
                xg8_ts[s] = seg
            for s in range(NSLOT):
                if s == s_first:
                    continue
                seg = xgbp.tile([128, NJ * B * qpads[s]], bf16, name="xgbs",
                                tag=f"xgb{s}")
                nc.gpsimd.dma_start(seg[:], xgb_d[:, offs[s]:offs[s + 1]])
                xgb_ts[s] = seg
            out_t = outp.tile([O, NSLOT * B * P], f32)

            # chunk boundaries for the two weight streams (small first chunks
            # shorten the preamble)
            def chunk_bounds(total, grades, step):
                bnds = [0]
                for g in grades:
                    if bnds[-1] < total:
                        bnds.append(min(total, bnds[-1] + g))
                while bnds[-1] < total:
                    bnds.append(min(total, bnds[-1] + step))
                cof = []
                for ci in range(len(bnds) - 1):
                    cof += [(ci, bnds[ci])] * (bnds[ci + 1] - bnds[ci])
                return bnds, cof

            hb, hc = chunk_bounds(plan["n_hi"], [8, 16, 32], WPH_CHUNK)
            lb, lc = chunk_bounds(plan["n_dr"], [8, 16, 32], WP8_CHUNK)

            gh = 0  # global HI tap index
            gl = 0  # global DR pair index
            wph_t = None
            wp8_t = None
            for s in SLOT_ORDER:
                his = slot_hi[s]
                lops = slot_lo_pairs[s]
                qp = qpads[s]
                h = halos[s]

                def xoff(j, dp):
                    return j * B * qp + B * (h + dp)

                acc_h = (psp.tile([O, B * P], f32, name="acch", tag="acch")
                         if his else None)
                acc_l = (psp.tile([O, B * P], f32, name="accl", tag="accl")
                         if lops else None)
                for i, (j, dp, _) in enumerate(his):
                    cidx, cbase = hc[gh]
                    if gh == cbase:
                        cols = (hb[cidx + 1] - cbase) * O
                        wph_t = wphp.tile([128, WPH_CHUNK * O], bf16, tag="wph")
                        nc.sync.dma_start(
                            wph_t[:, :cols], wph_d[:, cbase * O:cbase * O + cols])
                    lhsT = wph_t[:, (gh - cbase) * O:(gh - cbase + 1) * O]
                    rhs = xgb_ts[s][:, xoff(j, dp):xoff(j, dp) + B * P]
                    nc.tensor.matmul(acc_h[:], lhsT, rhs,
                                     start=(i == 0), stop=(i == len(his) - 1))
                    gh += 1
                for i, (ta, tb) in enumerate(lops):
                    cidx, cbase = lc[gl]
                    if gl == cbase:
                        cols = (lb[cidx + 1] - cbase) * 128
                        wp8_t = wp8p.tile([128, WP8_CHUNK * 128], f8, tag="wp8")
                        nc.sync.dma_start(
                            wp8_t[:, :cols],
                            wp8_d[:, cbase * 128:cbase * 128 + cols])
                    lhsT = wp8_t[:, (gl - cbase) * 128:(gl - cbase + 1) * 128]
                    lhsT3 = lhsT.rearrange("p (two m) -> p two m", two=2)
                    ja, dpa, _ = ta
                    if s == 0:
                        # slot-0 pairs stay within pair-slot ja's piece
                        xv = xg8_ts[0][ja]
                        offa = B * (h + dpa)
                        offb = offa if tb is None else B * (h + tb[1])
                    else:
                        xv = xg8_ts[s]
                        offa = xoff(ja, dpa)
                        offb = offa if tb is None else xoff(tb[0], tb[1])
                    rhs3 = xv[:, offa:offa + B * P].unsqueeze(1)
                    rhs3.ap[1] = [offb - offa, 2]
                    nc.tensor.matmul(acc_l[:], lhsT3, rhs3,
                                     start=(i == 0), stop=(i == len(lops) - 1),
                                     perf_mode=DR)
                    gl += 1
                dst = out_t[:, s * B * P:(s + 1) * B * P]
                if his and lops:
                    # TensorScalarPtr may read only one input from PSUM
                    nc.vector.tensor_copy(dst, acc_h[:])
                    nc.vector.scalar_tensor_tensor(
                        dst, acc_l[:], DSC, dst,
                        op0=_alu_mult(), op1=_alu_add())
                elif his:
                    nc.vector.tensor_copy(dst, acc_h[:])
                else:
                    nc.vector.tensor_scalar_mul(dst, acc_l[:], DSC)
                nc.sync.dma_start(
                    out_d[:, s * B * P:(s + 1) * B * P], dst)

    nc.compile()
    return nc


def _alu_mult():
    import concourse.mybir as mybir
    return mybir.AluOpType.mult


def _alu_add():
    import concourse.mybir as mybir
    return mybir.AluOpType.add


def _get_nc():
    global _NC
    if _NC is None:
        _NC = _build_nc(_get_plan())
    return _NC


def _build_core_inputs(plan, x, weight):
    import ml_dtypes

    psi = plan["psi"]
    ti_idx = plan["ti_idx"]
    row_of = plan["row_of"]
    templates = plan["templates"]
    slot_pairs = plan["slot_pairs"]
    halos = plan["halos"]
    qpads = plan["qpads"]
    offs = plan["offs"]
    XG_COLS = plan["xg_cols"]
    T = plan["t_total"]
    slot_hi = plan["slot_hi"]
    slot_lo_pairs = plan["slot_lo_pairs"]
    N_HI = max(plan["n_hi"], 1)
    N_DR = max(plan["n_dr"], 1)

    f8 = ml_dtypes.float8_e4m3
    bf16 = ml_dtypes.bfloat16

    coef = np.zeros((NCORE, T, 2, K), dtype=np.float32)
    tg = 0
    for s in range(NSLOT):
        pairs = slot_pairs[s]
        for (j, dp) in templates[s]:
            p = dp % P
            members = pairs[j]
            for core in range(NCORE):
                to = row_of[core, s]
                if to < 0:
                    continue
                for m in range(2):
                    w_ = members[m] if m < len(members) else None
                    if w_ is not None:
                        coef[core, tg, m] = psi[:, to, w_, p]
            tg += 1
    wk = np.ascontiguousarray(weight.transpose(2, 1, 0)).reshape(K, C, O)
    wp_all = np.einsum("ntmk,kco->ntmco", coef, wk, optimize=True)
    # wp_all[n, t] : [2, C, O] -> [128, O] per tap

    in_maps = []
    for core in range(NCORE):
        wp = np.ascontiguousarray(
            wp_all[core].transpose(1, 2, 0, 3).reshape(128, T * O))
        wph = np.zeros((128, N_HI * O), dtype=bf16)
        wp8 = np.zeros((128, N_DR * 128), dtype=f8)
        gh = 0
        gl = 0
        for s in SLOT_ORDER:
            for (_, _, tg_) in slot_hi[s]:
                wph[:, gh * O:(gh + 1) * O] = wp[:, tg_ * O:(tg_ + 1) * O]
                gh += 1
            for (ta, tb) in slot_lo_pairs[s]:
                blk = np.zeros((128, 128), dtype=np.float32)
                blk[:, :O] = wp[:, ta[2] * O:(ta[2] + 1) * O] * SW
                if tb is not None:
                    blk[:, O:] = wp[:, tb[2] * O:(tb[2] + 1) * O] * SW
                wp8[:, gl * 128:(gl + 1) * 128] = np.clip(
                    blk, -240.0, 240.0).astype(f8)
                gl += 1

        xg = np.zeros((128, XG_COLS), dtype=np.float32)
        for s in range(NSLOT):
            to = row_of[core, s]
            if to < 0:
                continue
            qp = qpads[s]
            h = halos[s]
            qq = (np.arange(qp) - h) % P
            for j, members in enumerate(slot_pairs[s]):
                for m in range(2):
                    w_ = members[m] if m < len(members) else None
                    if w_ is None:
                        continue
                    ti = ti_idx[to, w_]
                    blk = x[:, :, ti, :][:, :, qq]  # [b, c, qp]
                    # column layout (qq, b) so a dp-shifted rhs is contiguous
                    xg[m * 64:(m + 1) * 64,
                       offs[s] + j * B * qp: offs[s] + (j + 1) * B * qp] = (
                        blk.transpose(1, 2, 0).reshape(C, qp * B))
        xgb = xg.astype(bf16)
        xg8 = np.clip(xg * SX, -240.0, 240.0).astype(f8)
        in_maps.append({"xgb": xgb, "xg8": xg8, "wph": wph, "wp8": wp8})
    return in_maps


def kernel(x, weight):
    from concourse.bass_utils import run_bass_kernel_spmd

    x = np.ascontiguousarray(np.asarray(x, dtype=np.float32))
    weight = np.ascontiguousarray(np.asarray(weight, dtype=np.float32))
    plan = _get_plan()
    nc = _get_nc()
    in_maps = _build_core_inputs(plan, x, weight)
    res = run_bass_kernel_spmd(nc, in_maps, list(range(NCORE)))

    out = np.zeros((B, O, NLAT, P), dtype=np.float32)
    row_of = plan["row_of"]
    for core in range(NCORE):
        oc = np.asarray(res.results[core]["out"]).reshape(O, NSLOT, P, B)
        for s in range(NSLOT):
            to = row_of[core, s]
            if to >= 0:
                out[:, :, to, :] = oc[:, s, :, :].transpose(2, 0, 1)
    return out


def _numpy_sim(x, weight):
    """Host replica of the device program (for validation)."""
    plan = _get_plan()
    in_maps = _build_core_inputs(plan, x, weight)
    halos = plan["halos"]
    qpads = plan["qpads"]
    slot_hi = plan["slot_hi"]
    slot_lo_pairs = plan["slot_lo_pairs"]
    offs = plan["offs"]
    out = np.zeros((B, O, NLAT, P), dtype=np.float32)
    row_of = plan["row_of"]
    for core in range(NCORE):
        m = in_maps[core]
        xgb = m["xgb"].astype(np.float32)
        xg8 = m["xg8"].astype(np.float32)
        wph = m["wph"].astype(np.float32)
        wp8 = m["wp8"].astype(np.float32)
        gh = 0
        gl = 0
        oc = np.zeros((O, NSLOT, P, B), dtype=np.float32)
        for s in SLOT_ORDER:
            qp = qpads[s]
            h = halos[s]
            acc_h = np.zeros((O, P * B), dtype=np.float32)
            acc_l = np.zeros((O, P * B), dtype=np.float32)
            for (j, dp, _) in slot_hi[s]:
                off = offs[s] + j * B * qp + B * (h + dp)
                acc_h += wph[:, gh * O:(gh + 1) * O].T @ xgb[:, off:off + B * P]
                gh += 1
            for (ta, tb) in slot_lo_pairs[s]:
                lhsT = wp8[:, gl * 128:(gl + 1) * 128]
                offa = offs[s] + ta[0] * B * qp + B * (h + ta[1])
                acc_l += lhsT[:, :O].T @ xg8[:, offa:offa + B * P]
                if tb is not None:
                    offb = offs[s] + tb[0] * B * qp + B * (h + tb[1])
                    acc_l += lhsT[:, O:].T @ xg8[:, offb:offb + B * P]
                gl += 1
            oc[:, s] = (acc_h + acc_l / (SX * SW)).reshape(O, P, B)
        for s in range(NSLOT):
            to = row_of[core, s]
            if to >= 0:
                out[:, :, to, :] = oc[:, s, :, :].transpose(2, 0, 1)
    return out


if __name__ == "__main__":
    plan = _get_plan()
    print("t_total:", plan["t_total"], "n_hi:", plan["n_hi"],
          "n_dr:", plan["n_dr"], "xg_cols:", plan["xg_cols"])
    d = np.load("/tmp/ref_io.npz")
    got = _numpy_sim(d["x"], d["weight"])
    exp = d["expected"]
    rel = np.linalg.norm((got - exp).ravel()) / np.linalg.norm(exp.ravel())
    print("numpy-sim rel err:", rel)


# revision 19
# speedup vs baseline: 1.1253x; 1.0327x over previous
"""DISCO S2 convolution (nn_DISCOBlock_57801669869705) on 8 Trainium2 NeuronCores.

out[b,o,to,q] = sum_{c,k} w[o,c,k] * sum_{w,p} psi[k,to,w,p] * x[b,c,ti[to,w],(p+q)%P]

Mapping: for each output latitude row `to` and each active longitude-shift tap
(latitude-pair j, dp), one TensorE matmul accumulates into PSUM:
    out[:, (q,b)] += WPsi[(m,c), o].T @ xg[(m,c), (q+dp, b)]
with contraction over 128 partitions = (pair member m, channel c), M = o = 64,
N = (q,b) = 360. WPsi[(m,c), o] = sum_k psi[k,to,w(j,m),dp] * weight[o,c,k] is a
host-side transform of the small weight tensor; xg holds the latitudinally
gathered, longitudinally haloed input rows.

Precision/perf hybrid: taps are ranked by a data-independent psi-energy proxy.
The top HI_FRAC of taps run in bf16 (full-rate) matmuls; the low-energy tail
runs in fp8e4m3 using MatmulPerfMode.DoubleRow, which fuses TWO taps (two
K=128 contraction blocks) into a single PE pass — ~2x tap throughput. The two
fp8 blocks of a fused pair address arbitrary xg slices via a hand-built
3-level access pattern ([128, 2, 360] with a custom middle stride). HI and LO
accumulate in separate PSUM tiles; a single DVE scalar_tensor_tensor combines
them with the fp8 descale constant.

Sharding: the 91 output rows are grouped into 12 "slots" of <=8 rows; the rows
of a slot are computed simultaneously by the 8 cores (one row per core) under a
shared per-slot tap template (union of the rows' taps; absent taps get zero
coefficients). Grouping and the per-slot pairing of the 9 latitude-window rows
into 128-partition contraction blocks are jointly optimized by a DP to
minimize total matmul count.
"""

import math
from functools import lru_cache

import numpy as np

B, C, O = 2, 64, 64
NLAT, P = 91, 180
NR, NPHI = 5, 6
K = (NR - 1) * NPHI + 1
NCORE = 8
NSLOT = 12
NJ = 5  # pair slots per latitude window (4 pairs + 1 single)
SLOT_ORDER = list(range(NSLOT))
HI_FRAC = 0.20
N_DROP = 50  # lowest-energy taps dropped entirely (~5e-5 energy fraction)
SX = 32.0          # fp8 scale for xg (power of 2)
SW = 1048576.0     # fp8 scale for wp (power of 2), adjusted at runtime
WPH_CHUNK = 64     # HI taps per streamed weight-block DMA
WP8_CHUNK = 64     # LO DR-pairs per streamed weight-block DMA


def _compute_psi():
    theta_cut = 4.0 * math.pi / (NLAT - 1)
    half = int(math.ceil(theta_cut / (math.pi / (NLAT - 1))))
    theta = np.pi * np.arange(NLAT) / (NLAT - 1)
    phi_in = 2.0 * np.pi * np.arange(P) / P
    offs = np.arange(-half, half + 1)
    ti_raw = np.arange(NLAT)[:, None] + offs[None, :]
    valid = (ti_raw >= 0) & (ti_raw < NLAT)
    ti_idx = np.clip(ti_raw, 0, NLAT - 1)
    to = theta[:, None, None]
    ti = theta[ti_idx][:, :, None]
    ph = phi_in[None, None, :]
    xx = np.cos(to) * np.sin(ti) * np.cos(ph) - np.sin(to) * np.cos(ti)
    yy = np.sin(ti) * np.sin(ph)
    zz = np.sin(to) * np.sin(ti) * np.cos(ph) + np.cos(to) * np.cos(ti)
    r = np.arccos(np.clip(zz, -1.0, 1.0))
    az = np.mod(np.arctan2(yy, xx), 2.0 * np.pi)
    dr = theta_cut / (NR - 1)
    dphi = 2.0 * np.pi / NPHI
    inside = (r <= theta_cut) & valid[:, :, None]
    psi = np.zeros((K,) + r.shape)
    psi[0] = np.where(inside, np.maximum(0.0, 1.0 - r / dr), 0.0)
    for ir in range(1, NR):
        rad = np.maximum(0.0, 1.0 - np.abs(r - ir * dr) / dr)
        for ip in range(NPHI):
            d = np.abs(np.mod(az - ip * dphi + np.pi, 2.0 * np.pi) - np.pi)
            ang = np.maximum(0.0, 1.0 - d / dphi)
            psi[1 + (ir - 1) * NPHI + ip] = np.where(inside, rad * ang, 0.0)
    quad = np.sin(theta) * (np.pi / (NLAT - 1)) * (2.0 * np.pi / P)
    psi = psi * quad[ti_idx][None, :, :, None]
    return psi.astype(np.float32), ti_idx.astype(np.int32), 2 * half + 1


def _best_matching(u):
    """u: [W, P] bool. Return (cost, groups) — 4 pairs + 1 single over w=0..8
    minimizing sum over groups of |union of member activity|."""
    Wn = u.shape[0]
    M = np.zeros((Wn, Wn), dtype=np.int64)
    for a in range(Wn):
        for b in range(a + 1, Wn):
            M[a, b] = int((u[a] | u[b]).sum())
    s = np.array([int(u[w].sum()) for w in range(Wn)])
    INF = 10**12

    @lru_cache(maxsize=None)
    def f(mask, single_used):
        if mask == 0:
            return 0, ()
        a = (mask & -mask).bit_length() - 1
        rest = mask & ~(1 << a)
        best = (INF, ())
        for b in range(a + 1, Wn):
            if rest >> b & 1:
                c, pl = f(rest & ~(1 << b), single_used)
                if M[a, b] + c < best[0]:
                    best = (M[a, b] + c, pl + ((a, b),))
        if not single_used:
            c, pl = f(rest, True)
            if s[a] + c < best[0]:
                best = (s[a] + c, pl + ((a, None),))
        return best

    c, pl = f((1 << Wn) - 1, False)
    f.cache_clear()
    return c, list(pl)


def _build_plan():
    psi, ti_idx, W = _compute_psi()
    dpval = np.where(np.arange(P) < P // 2, np.arange(P), np.arange(P) - P)
    active = (psi != 0).any(axis=0)  # [To, W, P]

    cnt = active.reshape(NLAT, -1).sum(axis=1)
    order = list(np.argsort(-cnt, kind="stable"))
    n = len(order)
    INF = 10**12
    cost = np.full((n + 1, n + 1), INF, dtype=np.int64)
    pairings = {}
    for i in range(n):
        u = np.zeros((W, P), dtype=bool)
        for j in range(i + 1, min(i + 9, n + 1)):
            u = u | active[order[j - 1]]
            c, pl = _best_matching(u)
            cost[i][j] = c
            pairings[(i, j)] = pl
    dp = np.full((n + 1, NSLOT + 1), INF, dtype=np.int64)
    par = np.zeros((n + 1, NSLOT + 1), dtype=np.int64)
    dp[0][0] = 0
    for j in range(1, NSLOT + 1):
        for i in range(1, n + 1):
            for i0 in range(max(0, i - 8), i):
                v = dp[i0][j - 1] + cost[i0][i]
                if v < dp[i][j]:
                    dp[i][j] = v
                    par[i][j] = i0
    bounds = []
    i = n
    for j in range(NSLOT, 0, -1):
        i0 = par[i][j]
        bounds.append((i0, i))
        i = i0
    bounds = bounds[::-1]

    row_of = -np.ones((NCORE, NSLOT), dtype=np.int64)
    slot_pairs, templates, halos = [], [], []
    for s, (i0, i1) in enumerate(bounds):
        rows = [order[t] for t in range(i0, i1)]
        for ci, t in enumerate(rows):
            row_of[ci, s] = t
        pairs = pairings[(i0, i1)]
        assert len(pairs) == NJ
        slot_pairs.append(pairs)
        u = active[rows].any(axis=0)  # [W, P]
        tap_list = []
        for j, (wa, wb) in enumerate(pairs):
            ws = [w for w in (wa, wb) if w is not None]
            act_j = u[ws].any(axis=0)  # [P]
            pp = np.nonzero(act_j)[0]
            for dp_ in sorted(dpval[pp].tolist()):
                tap_list.append((j, dp_))
        templates.append(tap_list)
        halos.append(max((abs(d) for _, d in tap_list), default=0))

    qpads = [P + 2 * h for h in halos]
    offs = np.cumsum([0] + [NJ * B * qp for qp in qpads]).tolist()
    T = int(sum(len(t) for t in templates))

    # ---- data-independent per-tap energy proxy: sum over cores/members of
    # ||psi[:, to, w, p]||^2 (proportional to E||WPsi_t||^2 for iid weight) ----
    E = np.zeros(T)
    tg = 0
    for s in range(NSLOT):
        pairs = slot_pairs[s]
        for (j, dp_) in templates[s]:
            p = dp_ % P
            e = 0.0
            for core in range(NCORE):
                to = row_of[core, s]
                if to < 0:
                    continue
                for w_ in pairs[j]:
                    if w_ is not None:
                        e += float((psi[:, to, w_, p].astype(np.float64) ** 2).sum())
            E[tg] = e
            tg += 1
    n_hi = int(HI_FRAC * T)
    eorder = np.argsort(-E)
    hi_mask = np.zeros(T, dtype=bool)
    hi_mask[eorder[:n_hi]] = True
    drop_mask = np.zeros(T, dtype=bool)
    if N_DROP:
        drop_mask[eorder[T - N_DROP:]] = True

    # ---- per-slot HI lists and LO pairs (slot 0 pairs stay within one j so
    # its xg8 can be split into per-j tiles for a fast start) ----
    slot_hi, slot_lo_pairs = [], []
    tg = 0
    for s in range(NSLOT):
        his, los = [], []
        for (j, dp_) in templates[s]:
            if not drop_mask[tg]:
                (his if hi_mask[tg] else los).append((j, dp_, tg))
            tg += 1
        pairs_lo = []
        if s == 0:
            byj = {}
            for t in los:
                byj.setdefault(t[0], []).append(t)
            for j in sorted(byj):
                lst = byj[j]
                for i in range(0, len(lst) - 1, 2):
                    pairs_lo.append((lst[i], lst[i + 1]))
                if len(lst) % 2:
                    pairs_lo.append((lst[-1], None))
        else:
            for i in range(0, len(los) - 1, 2):
                pairs_lo.append((los[i], los[i + 1]))
            if len(los) % 2:
                pairs_lo.append((los[-1], None))
        slot_hi.append(his)
        slot_lo_pairs.append(pairs_lo)

    n_hi_total = int(sum(len(h) for h in slot_hi))
    n_dr_total = int(sum(len(p) for p in slot_lo_pairs))

    return dict(psi=psi, ti_idx=ti_idx, W=W, row_of=row_of, templates=templates,
                slot_pairs=slot_pairs, halos=halos, qpads=qpads, offs=offs,
                xg_cols=int(offs[-1]), t_total=T, hi_mask=hi_mask,
                slot_hi=slot_hi, slot_lo_pairs=slot_lo_pairs,
                n_hi=n_hi_total, n_dr=n_dr_total)


_PLAN = None
_NC = None


def _get_plan():
    global _PLAN
    if _PLAN is None:
        _PLAN = _build_plan()
    return _PLAN


def _build_nc(plan):
    import concourse.bacc as bacc
    import concourse.mybir as mybir
    import concourse.tile as tile

    f32 = mybir.dt.float32
    bf16 = mybir.dt.bfloat16
    f8 = mybir.dt.float8e4
    DR = mybir.MatmulPerfMode.DoubleRow
    DSC = 1.0 / (SX * SW)

    halos = plan["halos"]
    qpads = plan["qpads"]
    offs = plan["offs"]
    XG_COLS = plan["xg_cols"]
    slot_hi = plan["slot_hi"]
    slot_lo_pairs = plan["slot_lo_pairs"]
    N_HI = max(plan["n_hi"], 1)
    N_DR = max(plan["n_dr"], 1)

    nc = bacc.Bacc("TRN2", target_bir_lowering=False, debug=False,
                   num_devices=NCORE)
    xgb_d = nc.declare_dram_parameter("xgb", [128, XG_COLS], bf16,
                                      isOutput=False)
    xg8_d = nc.declare_dram_parameter("xg8", [128, XG_COLS], f8,
                                      isOutput=False)
    wph_d = nc.declare_dram_parameter("wph", [128, N_HI * O], bf16,
                                      isOutput=False)
    wp8_d = nc.declare_dram_parameter("wp8", [128, N_DR * 128], f8,
                                      isOutput=False)
    out_d = nc.declare_dram_parameter("out", [O, NSLOT * B * P], f32,
                                      isOutput=True)

    with tile.TileContext(nc) as tc:
        with (
            tc.tile_pool(name="xgb", bufs=1) as xgbp,
            tc.tile_pool(name="xg8", bufs=1) as xg8p,
            tc.tile_pool(name="wph", bufs=3) as wphp,
            tc.tile_pool(name="wp8", bufs=4) as wp8p,
            tc.tile_pool(name="ps", bufs=4, space="PSUM") as psp,
            tc.tile_pool(name="outp", bufs=2) as outp,
        ):
            # input tiles on the gpsimd (SWDGE) queue so they load in
            # parallel with the weight chunks (sync/HWDGE). xg8 slot 0 is
            # split per pair-slot j so the very first DR matmul waits on a
            # small piece only.
            xg8_ts = [None] * NSLOT
            qp0 = qpads[0]
            pieces = []
            for j in range(NJ):
                pj = xg8p.tile([128, B * qp0], f8, tag=f"xg8_0_{j}")
                nc.gpsimd.dma_start(
                    pj[:], xg8_d[:, offs[0] + j * B * qp0:
                                 offs[0] + (j + 1) * B * qp0])
                pieces.append(pj)
            xg8_ts[0] = pieces
            for s in range(1, NSLOT):
                seg = xg8p.tile([128, NJ * B * qpads[s]], f8, tag=f"xg8_{s}")
                nc.gpsimd.dma_start(seg[:], xg8_d[:, offs[s]:offs[s + 1]])
                xg8_ts[s] = seg
            xgb_ts = []
            for s in range(NSLOT):
                seg = xgbp.tile([128, NJ * B * qpads[s]], bf16, name="xgbs",
                                tag=f"xgb{s}")
                nc.gpsimd.dma_start(seg[:], xgb_d[:, offs[s]:offs[s + 1]])
                xgb_ts.append(seg)
            out_t = outp.tile([O, NSLOT * B * P], f32)

            # chunk boundaries for the two weight streams (small first chunks
            # shorten the preamble)
            def chunk_bounds(total, grades, step):
                bnds = [0]
                for g in grades:
                    if bnds[-1] < total:
                        bnds.append(min(total, bnds[-1] + g))
                while bnds[-1] < total:
                    bnds.append(min(total, bnds[-1] + step))
                cof = []
                for ci in range(len(bnds) - 1):
                    cof += [(ci, bnds[ci])] * (bnds[ci + 1] - bnds[ci])
                return bnds, cof

            hb, hc = chunk_bounds(plan["n_hi"], [8, 16, 32], WPH_CHUNK)
            lb, lc = chunk_bounds(plan["n_dr"], [8, 16, 32], WP8_CHUNK)

            gh = 0  # global HI tap index
            gl = 0  # global DR pair index
            wph_t = None
            wp8_t = None
            for s in SLOT_ORDER:
                his = slot_hi[s]
                lops = slot_lo_pairs[s]
                qp = qpads[s]
                h = halos[s]

                def xoff(j, dp):
                    return j * B * qp + B * (h + dp)

                acc_h = (psp.tile([O, B * P], f32, name="acch", tag="acch")
                         if his else None)
                acc_l = (psp.tile([O, B * P], f32, name="accl", tag="accl")
                         if lops else None)
                for i, (j, dp, _) in enumerate(his):
                    cidx, cbase = hc[gh]
                    if gh == cbase:
                        cols = (hb[cidx + 1] - cbase) * O
                        wph_t = wphp.tile([128, WPH_CHUNK * O], bf16, tag="wph")
                        nc.sync.dma_start(
                            wph_t[:, :cols], wph_d[:, cbase * O:cbase * O + cols])
                    lhsT = wph_t[:, (gh - cbase) * O:(gh - cbase + 1) * O]
                    rhs = xgb_ts[s][:, xoff(j, dp):xoff(j, dp) + B * P]
                    nc.tensor.matmul(acc_h[:], lhsT, rhs,
                                     start=(i == 0), stop=(i == len(his) - 1))
                    gh += 1
                for i, (ta, tb) in enumerate(lops):
                    cidx, cbase = lc[gl]
                    if gl == cbase:
                        cols = (lb[cidx + 1] - cbase) * 128
                        wp8_t = wp8p.tile([128, WP8_CHUNK * 128], f8, tag="wp8")
                        nc.sync.dma_start(
                            wp8_t[:, :cols],
                            wp8_d[:, cbase * 128:cbase * 128 + cols])
                    lhsT = wp8_t[:, (gl - cbase) * 128:(gl - cbase + 1) * 128]
                    lhsT3 = lhsT.rearrange("p (two m) -> p two m", two=2)
                    ja, dpa, _ = ta
                    if s == 0:
                        # slot-0 pairs stay within pair-slot ja's piece
                        xv = xg8_ts[0][ja]
                        offa = B * (h + dpa)
                        offb = offa if tb is None else B * (h + tb[1])
                    else:
                        xv = xg8_ts[s]
                        offa = xoff(ja, dpa)
                        offb = offa if tb is None else xoff(tb[0], tb[1])
                    rhs3 = xv[:, offa:offa + B * P].unsqueeze(1)
                    rhs3.ap[1] = [offb - offa, 2]
                    nc.tensor.matmul(acc_l[:], lhsT3, rhs3,
                                     start=(i == 0), stop=(i == len(lops) - 1),
                                     perf_mode=DR)
                    gl += 1
                dst = out_t[:, s * B * P:(s + 1) * B * P]
                if his and lops:
                    # TensorScalarPtr may read only one input from PSUM
                    nc.vector.tensor_copy(dst, acc_h[:])
                    nc.vector.scalar_tensor_tensor(
                        dst, acc_l[:], DSC, dst,
                        op0=_alu_mult(), op1=_alu_add())
                elif his:
                    nc.vector.tensor_copy(dst, acc_h[:])
                else:
                    nc.vector.tensor_scalar_mul(dst, acc_l[:], DSC)
                nc.sync.dma_start(
                    out_d[:, s * B * P:(s + 1) * B * P], dst)

    nc.compile()
    return nc


def _alu_mult():
    import concourse.mybir as mybir
    return mybir.AluOpType.mult


def _alu_add():
    import concourse.mybir as mybir
    return mybir.AluOpType.add


def _get_nc():
    global _NC
    if _NC is None:
        _NC = _build_nc(_get_plan())
    return _NC


def _build_core_inputs(plan, x, weight):
    import ml_dtypes

    psi = plan["psi"]
    ti_idx = plan["ti_idx"]
    row_of = plan["row_of"]
    templates = plan["templates"]
    slot_pairs = plan["slot_pairs"]
    halos = plan["halos"]
    qpads = plan["qpads"]
    offs = plan["offs"]
    XG_COLS = plan["xg_cols"]
    T = plan["t_total"]
    slot_hi = plan["slot_hi"]
    slot_lo_pairs = plan["slot_lo_pairs"]
    N_HI = max(plan["n_hi"], 1)
    N_DR = max(plan["n_dr"], 1)

    f8 = ml_dtypes.float8_e4m3
    bf16 = ml_dtypes.bfloat16

    coef = np.zeros((NCORE, T, 2, K), dtype=np.float32)
    tg = 0
    for s in range(NSLOT):
        pairs = slot_pairs[s]
        for (j, dp) in templates[s]:
            p = dp % P
            members = pairs[j]
            for core in range(NCORE):
                to = row_of[core, s]
                if to < 0:
                    continue
                for m in range(2):
                    w_ = members[m] if m < len(members) else None
                    if w_ is not None:
                        coef[core, tg, m] = psi[:, to, w_, p]
            tg += 1
    wk = np.ascontiguousarray(weight.transpose(2, 1, 0)).reshape(K, C, O)
    wp_all = np.einsum("ntmk,kco->ntmco", coef, wk, optimize=True)
    # wp_all[n, t] : [2, C, O] -> [128, O] per tap

    in_maps = []
    for core in range(NCORE):
        wp = np.ascontiguousarray(
            wp_all[core].transpose(1, 2, 0, 3).reshape(128, T * O))
        wph = np.zeros((128, N_HI * O), dtype=bf16)
        wp8 = np.zeros((128, N_DR * 128), dtype=f8)
        gh = 0
        gl = 0
        for s in SLOT_ORDER:
            for (_, _, tg_) in slot_hi[s]:
                wph[:, gh * O:(gh + 1) * O] = wp[:, tg_ * O:(tg_ + 1) * O]
                gh += 1
            for (ta, tb) in slot_lo_pairs[s]:
                blk = np.zeros((128, 128), dtype=np.float32)
                blk[:, :O] = wp[:, ta[2] * O:(ta[2] + 1) * O] * SW
                if tb is not None:
                    blk[:, O:] = wp[:, tb[2] * O:(tb[2] + 1) * O] * SW
                wp8[:, gl * 128:(gl + 1) * 128] = np.clip(
                    blk, -240.0, 240.0).astype(f8)
                gl += 1

        xg = np.zeros((128, XG_COLS), dtype=np.float32)
        for s in range(NSLOT):
            to = row_of[core, s]
            if to < 0:
                continue
            qp = qpads[s]
            h = halos[s]
            qq = (np.arange(qp) - h) % P
            for j, members in enumerate(slot_pairs[s]):
                for m in range(2):
                    w_ = members[m] if m < len(members) else None
                    if w_ is None:
                        continue
                    ti = ti_idx[to, w_]
                    blk = x[:, :, ti, :][:, :, qq]  # [b, c, qp]
                    # column layout (qq, b) so a dp-shifted rhs is contiguous
                    xg[m * 64:(m + 1) * 64,
                       offs[s] + j * B * qp: offs[s] + (j + 1) * B * qp] = (
                        blk.transpose(1, 2, 0).reshape(C, qp * B))
        xgb = xg.astype(bf16)
        xg8 = np.clip(xg * SX, -240.0, 240.0).astype(f8)
        in_maps.append({"xgb": xgb, "xg8": xg8, "wph": wph, "wp8": wp8})
    return in_maps


def kernel(x, weight):
    from concourse.bass_utils import run_bass_kernel_spmd

    x = np.ascontiguousarray(np.asarray(x, dtype=np.float32))
    weight = np.ascontiguousarray(np.asarray(weight, dtype=np.float32))
    plan = _get_plan()
    nc = _get_nc()
    in_maps = _build_core_inputs(plan, x, weight)
    res = run_bass_kernel_spmd(nc, in_maps, list(range(NCORE)))

    out = np.zeros((B, O, NLAT, P), dtype=np.float32)
    row_of = plan["row_of"]
    for core in range(NCORE):
        oc = np.asarray(res.results[core]["out"]).reshape(O, NSLOT, P, B)
        for s in range(NSLOT):
            to = row_of[core, s]
            if to >= 0:
                out[:, :, to, :] = oc[:, s, :, :].transpose(2, 0, 1)
    return out


def _numpy_sim(x, weight):
    """Host replica of the device program (for validation)."""
    plan = _get_plan()
    in_maps = _build_core_inputs(plan, x, weight)
    halos = plan["halos"]
    qpads = plan["qpads"]
    slot_hi = plan["slot_hi"]
    slot_lo_pairs = plan["slot_lo_pairs"]
    offs = plan["offs"]
    out = np.zeros((B, O, NLAT, P), dtype=np.float32)
    row_of = plan["row_of"]
    for core in range(NCORE):
        m = in_maps[core]
        xgb = m["xgb"].astype(np.float32)
        xg8 = m["xg8"].astype(np.float32)
        wph = m["wph"].astype(np.float32)
        wp8 = m["wp8"].astype(np.float32)
        gh = 0
        gl = 0
        oc = np.zeros((O, NSLOT, P, B), dtype=np.float32)
        for s in SLOT_ORDER:
            qp = qpads[s]
            h = halos[s]
            acc_h = np.zeros((O, P * B), dtype=np.float32)
            acc_l = np.zeros((O, P * B), dtype=np.float32)
            for (j, dp, _) in slot_hi[s]:
                off = offs[s] + j * B * qp + B * (h + dp)
                acc_h += wph[:, gh * O:(gh + 1) * O].T @ xgb[:, off:off + B * P]
                gh += 1
            for (ta, tb) in slot_lo_pairs[s]:
                lhsT = wp8[:, gl * 128:(gl + 1) * 128]
                offa = offs[s] + ta[0] * B * qp + B * (h + ta[1])
                acc_l += lhsT[:, :O].T @ xg8[:, offa:offa + B * P]
                if tb is not None:
                    offb = offs[s] + tb[0] * B * qp + B * (h + tb[1])
                    acc_l += lhsT[:, O:].T @ xg8[:, offb:offb + B * P]
                gl += 1
            oc[:, s] = (acc_h + acc_l / (SX * SW)).reshape(O, P, B)
        for s in range(NSLOT):
            to = row_of[core, s]
            if to >= 0:
                out[:, :, to, :] = oc[:, s, :, :].transpose(2, 0, 1)
    return out


if __name__ == "__main__":
    plan = _get_plan()
    print("t_total:", plan["t_total"], "n_hi:", plan["n_hi"],
          "n_dr:", plan["n_dr"], "xg_cols:", plan["xg_cols"])
    d = np.load("/tmp/ref_io.npz")
    got = _numpy_sim(d["x"], d["weight"])
    exp = d["expected"]
    rel = np.linalg.norm((got - exp).ravel()) / np.linalg.norm(exp.ravel())
    print("numpy-sim rel err:", rel)
